# revision 7
# baseline (speedup 1.0000x reference)
"""Trainium2 Bass kernel for nn_MessagePassing_9887014715655 (gnn_message_passing).

Reference computes:
    target   = edge_index[1]
    messages = x[target] * W[:, None]          # gather on target
    aggr     = segment_sum(messages, target)   # scatter on the SAME target

Because the gather index and the scatter index are identical, every message
for node n is x[n] * W[e], so

    aggr[n] = x[n] * s[n],   s = segment_sum(W, target)   # [N] weighted degree

The kernel therefore needs a weighted histogram of W over targets plus an
elementwise scale of x — a purely memory-bound problem (target_regime=memory).

Distribution strategy (chosen; the hint's edge-parallel+allreduce is strictly
worse here): the host performs LAYOUT ONLY — it stable-sorts edges by target
and scatters W into a zero-padded dense per-node matrix Wpad[n, :deg(n)], then
range-shards nodes across the 8 cores.  ALL floating-point arithmetic (the
per-node segment sums and the x*s scale) happens on device: each core
row-reduces its Wpad shard and multiplies its x shard by the result.
Node-range sharding makes each core's output independent, so no collective is
needed; each core streams Wpad (2.0MB) + x (1.6MB) in and out (1.6MB) at HBM
rate, double-buffered so DVE compute hides under DMA.
"""

import numpy as np

import concourse.bass as bass
import concourse.mybir as mybir
from concourse.bass_utils import run_bass_kernel_spmd

P = 128            # SBUF partitions
D = 32             # feature dim
N_CORES = 8
N_NODES = 100000
G = 98             # node-column groups per core; P*G*N_CORES = 100352 >= N_NODES
NPC = P * G        # nodes per core (12544)
N_PAD = NPC * N_CORES
F32 = mybir.dt.float32

_cache: dict = {}


def _build(K: int, n_chunks: int):
    """Per-core SPMD program (raw bass, manual semaphores).

    DRAM inputs (host pre-permuted so each partition's data is contiguous):
      wpad [P, G*K]  partition p, column-group g holds W-pad row of node g*P+p
      xin  [P, G*D]  same node layout for x
    Output:
      out  [P, G*D]  out[p, g*D:(g+1)*D] = xin[p, ...] * sum(wpad[p, g*K:(g+1)*K])
    """
    key = (K, n_chunks)
    if key in _cache:
        return _cache[key]

    nc = bass.Bass()
    wpad = nc.declare_dram_parameter("wpad", [P, G * K], F32, isOutput=False)
    xin = nc.declare_dram_parameter("xin", [P, G * D], F32, isOutput=False)
    out = nc.declare_dram_parameter("out", [P, G * D], F32, isOutput=True)

    CH = n_chunks
    base = G // CH
    sizes = [base + (1 if i < G % CH else 0) for i in range(CH)]
    offs = [sum(sizes[:i]) for i in range(CH)]
    gmax = max(sizes)

    import contextlib

    with contextlib.ExitStack() as ctx:
        wbuf = [
            ctx.enter_context(nc.sbuf_tensor(f"wbuf{j}", [P, gmax * K], F32))
            for j in range(2)
        ]
        xbuf = [
            ctx.enter_context(nc.sbuf_tensor(f"xbuf{j}", [P, gmax * D], F32))
            for j in range(2)
        ]
        obuf = [
            ctx.enter_context(nc.sbuf_tensor(f"obuf{j}", [P, gmax * D], F32))
            for j in range(2)
        ]
        sbuf = [
            ctx.enter_context(nc.sbuf_tensor(f"sbuf{j}", [P, gmax], F32))
            for j in range(2)
        ]
        # Separate sems per input stream: a shared sem would be racy — the 16
        # SDMA engines' +1 completions from concurrent DMAs interleave, so a
        # combined count can hit 32i+16 before the w-load fully landed.
        dinw = ctx.enter_context(nc.semaphore("dinw"))
        dinx = ctx.enter_context(nc.semaphore("dinx"))
        dout = ctx.enter_context(nc.semaphore("dout"))
        vd = ctx.enter_context(nc.semaphore("vd"))
        block = ctx.enter_context(nc.Block())

        @block.sync
        def _(sync):
            # stream: L0, L1, S0, L2, S1, ..., L_{CH-1}, S_{CH-2}, S_{CH-1}
            def load(i):
                b = i % 2
                gc, g0 = sizes[i], offs[i]
                if i >= 2:
                    # WAR: wbuf/xbuf[b] are free once chunk i-2's mult retired
                    sync.wait_ge(vd, i - 1)
                sync.dma_start(
                    out=wbuf[b][:, : gc * K], in_=wpad[:, g0 * K:(g0 + gc) * K]
                ).then_inc(dinw, 16)
                sync.dma_start(
                    out=xbuf[b][:, : gc * D], in_=xin[:, g0 * D:(g0 + gc) * D]
                ).then_inc(dinx, 16)

            def store(i):
                b = i % 2
                gc, g0 = sizes[i], offs[i]
                sync.wait_ge(vd, i + 1)
                sync.dma_start(
                    out=out[:, g0 * D:(g0 + gc) * D], in_=obuf[b][:, : gc * D]
                ).then_inc(dout, 16)

            load(0)
            if CH > 1:
                load(1)
            for i in range(CH):
                if i + 2 < CH:
                    load(i + 2)
                store(i)
            sync.wait_ge(dout, 16 * CH)

        @block.vector
        def _(vector):
            for i in range(CH):
                b = i % 2
                gc = sizes[i]
                vector.wait_ge(dinw, 16 * i + 16)  # w_i fully landed
                vector.tensor_reduce(
                    out=sbuf[b][:, :gc],
                    in_=wbuf[b][:, : gc * K].rearrange("p (g k) -> p g k", k=K),
                    axis=mybir.AxisListType.X,
                    op=mybir.AluOpType.add,
                )
                vector.wait_ge(dinx, 16 * i + 16)  # x_i fully landed
                if i >= 2:
                    vector.wait_ge(dout, 16 * (i - 1))  # obuf[b] stored out
                vector.tensor_tensor(
                    out=obuf[b][:, : gc * D].rearrange("p (g d) -> p g d", d=D),
                    in0=xbuf[b][:, : gc * D].rearrange("p (g d) -> p g d", d=D),
                    in1=sbuf[b][:, :gc].unsqueeze(2).to_broadcast([P, gc, D]),
                    op=mybir.AluOpType.mult,
                ).then_inc(vd, 1)

    _cache[key] = nc
    return nc


def _shard(a_padded: np.ndarray, width: int, core: int) -> np.ndarray:
    """[N_PAD, width] row-major -> this core's [P, G*width] partition-major."""
    sl = a_padded[core * NPC:(core + 1) * NPC]
    return np.ascontiguousarray(
        sl.reshape(G, P, width).transpose(1, 0, 2).reshape(P, G * width)
    )


def _prep(edge_index, x, W):
    """Host-side layout (integer metadata + data movement only, no FP math)."""
    t = np.asarray(edge_index)[1].astype(np.int64)
    x = np.ascontiguousarray(np.asarray(x, dtype=np.float32))
    W = np.ascontiguousarray(np.asarray(W, dtype=np.float32))
    n_nodes = x.shape[0]
    assert n_nodes <= N_PAD and x.shape[1] == D

    cnt = np.bincount(t, minlength=N_PAD)
    K = max(40, int(-(-int(cnt.max()) // 8) * 8))
    order = np.argsort(t, kind="stable")
    ts = t[order]
    Ws = W[order]
    starts = np.zeros(N_PAD, dtype=np.int64)
    starts[1:] = np.cumsum(cnt)[:-1]
    pos = np.arange(ts.shape[0], dtype=np.int64) - starts[ts]
    Wpad = np.zeros((N_PAD, K), dtype=np.float32)
    Wpad[ts, pos] = Ws
    xpad = np.zeros((N_PAD, D), dtype=np.float32)
    xpad[:n_nodes] = x

    in_maps = [
        {"wpad": _shard(Wpad, K, c), "xin": _shard(xpad, D, c)}
        for c in range(N_CORES)
    ]
    return in_maps, K, n_nodes


def _assemble(results, n_nodes):
    outs = []
    for c in range(N_CORES):
        oc = results[c]["out"].reshape(P, G, D).transpose(1, 0, 2)
        outs.append(oc.reshape(NPC, D))
    full = np.concatenate(outs, axis=0)[:n_nodes]
    return np.ascontiguousarray(full, dtype=np.float32)


def _run(edge_index, x, W, trace=False, n_chunks=2):
    in_maps, K, n_nodes = _prep(edge_index, x, W)
    nc = _build(K, n_chunks)
    res = run_bass_kernel_spmd(nc, in_maps, list(range(N_CORES)), trace=trace)
    return _assemble(res.results, n_nodes), res


def kernel(edge_index, x, W):
    out, _ = _run(edge_index, x, W)
    return out


# revision 9
# speedup vs baseline: 1.0288x; 1.0288x over previous
"""Trainium2 Bass kernel for nn_MessagePassing_9887014715655 (gnn_message_passing).

Reference computes:
    target   = edge_index[1]
    messages = x[target] * W[:, None]          # gather on target
    aggr     = segment_sum(messages, target)   # scatter on the SAME target

Because the gather index and the scatter index are identical, every message
for node n is x[n] * W[e], so

    aggr[n] = x[n] * s[n],   s = segment_sum(W, target)   # [N] weighted degree

The kernel therefore needs a weighted histogram of W over targets plus an
elementwise scale of x — a purely memory-bound problem (target_regime=memory).

Distribution strategy (chosen; the hint's edge-parallel+allreduce is strictly
worse here): the host performs LAYOUT ONLY — it stable-sorts edges by target
and scatters W into a zero-padded dense per-node matrix Wpad[n, :deg(n)], then
range-shards nodes across the 8 cores.  ALL floating-point arithmetic (the
per-node segment sums and the x*s scale) happens on device: each core
row-reduces its Wpad shard and multiplies its x shard by the result.
Node-range sharding makes each core's output independent, so no collective is
needed; each core streams Wpad (2.0MB) + x (1.6MB) in and out (1.6MB) at HBM
rate, double-buffered so DVE compute hides under DMA.
"""

import numpy as np

import concourse.bass as bass
import concourse.mybir as mybir
from concourse.bass_utils import run_bass_kernel_spmd

P = 128            # SBUF partitions
D = 32             # feature dim
N_CORES = 8
N_NODES = 100000
G = 98             # node-column groups per core; P*G*N_CORES = 100352 >= N_NODES
NPC = P * G        # nodes per core (12544)
N_PAD = NPC * N_CORES
F32 = mybir.dt.float32

_cache: dict = {}


def _build(K: int, n_chunks: int):
    """Per-core SPMD program (raw bass, manual semaphores).

    DRAM inputs (host pre-permuted so each partition's data is contiguous):
      wpad [P, G*K]  partition p, column-group g holds W-pad row of node g*P+p
      xin  [P, G*D]  same node layout for x
    Output:
      out  [P, G*D]  out[p, g*D:(g+1)*D] = xin[p, ...] * sum(wpad[p, g*K:(g+1)*K])
    """
    key = (K, n_chunks)
    if key in _cache:
        return _cache[key]

    # Skip the module-init all-engine EVSEM barrier (~3us): our first DMA
    # (HWDGE on SP) has no dependency on the Pool const-memsets it fences.
    _orig_barrier = bass.Bass.all_engine_barrier
    try:
        bass.Bass.all_engine_barrier = lambda self, **kw: None
        nc = bass.Bass()
    finally:
        bass.Bass.all_engine_barrier = _orig_barrier
    wpad = nc.declare_dram_parameter("wpad", [P, G * K], F32, isOutput=False)
    xin = nc.declare_dram_parameter("xin", [P, G * D], F32, isOutput=False)
    out = nc.declare_dram_parameter("out", [P, G * D], F32, isOutput=True)

    CH = n_chunks
    base = G // CH
    sizes = [base + (1 if i < G % CH else 0) for i in range(CH)]
    offs = [sum(sizes[:i]) for i in range(CH)]
    gmax = max(sizes)

    import contextlib

    with contextlib.ExitStack() as ctx:
        wbuf = [
            ctx.enter_context(nc.sbuf_tensor(f"wbuf{j}", [P, gmax * K], F32))
            for j in range(2)
        ]
        xbuf = [
            ctx.enter_context(nc.sbuf_tensor(f"xbuf{j}", [P, gmax * D], F32))
            for j in range(2)
        ]
        obuf = [
            ctx.enter_context(nc.sbuf_tensor(f"obuf{j}", [P, gmax * D], F32))
            for j in range(2)
        ]
        sbuf = [
            ctx.enter_context(nc.sbuf_tensor(f"sbuf{j}", [P, gmax], F32))
            for j in range(2)
        ]
        # Separate sems per input stream: a shared sem would be racy — the 16
        # SDMA engines' +1 completions from concurrent DMAs interleave, so a
        # combined count can hit 32i+16 before the w-load fully landed.
        dinw = ctx.enter_context(nc.semaphore("dinw"))
        dinx = ctx.enter_context(nc.semaphore("dinx"))
        dout = ctx.enter_context(nc.semaphore("dout"))
        vd = ctx.enter_context(nc.semaphore("vd"))
        block = ctx.enter_context(nc.Block(no_gpsimd_drain=True))

        @block.sync
        def _(sync):
            # stream: L0, L1, S0, L2, S1, ..., L_{CH-1}, S_{CH-2}, S_{CH-1}
            def load(i):
                b = i % 2
                gc, g0 = sizes[i], offs[i]
                if i >= 2:
                    # WAR: wbuf/xbuf[b] are free once chunk i-2's mult retired
                    sync.wait_ge(vd, i - 1)
                sync.dma_start(
                    out=wbuf[b][:, : gc * K], in_=wpad[:, g0 * K:(g0 + gc) * K]
                ).then_inc(dinw, 16)
                sync.dma_start(
                    out=xbuf[b][:, : gc * D], in_=xin[:, g0 * D:(g0 + gc) * D]
                ).then_inc(dinx, 16)

            def store(i):
                b = i % 2
                gc, g0 = sizes[i], offs[i]
                sync.wait_ge(vd, i + 1)
                sync.dma_start(
                    out=out[:, g0 * D:(g0 + gc) * D], in_=obuf[b][:, : gc * D]
                ).then_inc(dout, 16)

            load(0)
            if CH > 1:
                load(1)
            for i in range(CH):
                if i + 2 < CH:
                    load(i + 2)
                store(i)
            sync.wait_ge(dout, 16 * CH)

        @block.vector
        def _(vector):
            for i in range(CH):
                b = i % 2
                gc = sizes[i]
                vector.wait_ge(dinw, 16 * i + 16)  # w_i fully landed
                vector.tensor_reduce(
                    out=sbuf[b][:, :gc],
                    in_=wbuf[b][:, : gc * K].rearrange("p (g k) -> p g k", k=K),
                    axis=mybir.AxisListType.X,
                    op=mybir.AluOpType.add,
                )
                vector.wait_ge(dinx, 16 * i + 16)  # x_i fully landed
                if i >= 2:
                    vector.wait_ge(dout, 16 * (i - 1))  # obuf[b] stored out
                vector.tensor_tensor(
                    out=obuf[b][:, : gc * D].rearrange("p (g d) -> p g d", d=D),
                    in0=xbuf[b][:, : gc * D].rearrange("p (g d) -> p g d", d=D),
                    in1=sbuf[b][:, :gc].unsqueeze(2).to_broadcast([P, gc, D]),
                    op=mybir.AluOpType.mult,
                ).then_inc(vd, 1)

    _cache[key] = nc
    return nc


def _shard(a_padded: np.ndarray, width: int, core: int) -> np.ndarray:
    """[N_PAD, width] row-major -> this core's [P, G*width] partition-major."""
    sl = a_padded[core * NPC:(core + 1) * NPC]
    return np.ascontiguousarray(
        sl.reshape(G, P, width).transpose(1, 0, 2).reshape(P, G * width)
    )


def _prep(edge_index, x, W):
    """Host-side layout (integer metadata + data movement only, no FP math)."""
    t = np.asarray(edge_index)[1].astype(np.int64)
    x = np.ascontiguousarray(np.asarray(x, dtype=np.float32))
    W = np.ascontiguousarray(np.asarray(W, dtype=np.float32))
    n_nodes = x.shape[0]
    assert n_nodes <= N_PAD and x.shape[1] == D

    cnt = np.bincount(t, minlength=N_PAD)
    K = max(40, int(-(-int(cnt.max()) // 8) * 8))
    order = np.argsort(t, kind="stable")
    ts = t[order]
    Ws = W[order]
    starts = np.zeros(N_PAD, dtype=np.int64)
    starts[1:] = np.cumsum(cnt)[:-1]
    pos = np.arange(ts.shape[0], dtype=np.int64) - starts[ts]
    Wpad = np.zeros((N_PAD, K), dtype=np.float32)
    Wpad[ts, pos] = Ws
    xpad = np.zeros((N_PAD, D), dtype=np.float32)
    xpad[:n_nodes] = x

    in_maps = [
        {"wpad": _shard(Wpad, K, c), "xin": _shard(xpad, D, c)}
        for c in range(N_CORES)
    ]
    return in_maps, K, n_nodes


def _assemble(results, n_nodes):
    outs = []
    for c in range(N_CORES):
        oc = results[c]["out"].reshape(P, G, D).transpose(1, 0, 2)
        outs.append(oc.reshape(NPC, D))
    full = np.concatenate(outs, axis=0)[:n_nodes]
    return np.ascontiguousarray(full, dtype=np.float32)


def _run(edge_index, x, W, trace=False, n_chunks=2):
    in_maps, K, n_nodes = _prep(edge_index, x, W)
    nc = _build(K, n_chunks)
    res = run_bass_kernel_spmd(nc, in_maps, list(range(N_CORES)), trace=trace)
    return _assemble(res.results, n_nodes), res


def kernel(edge_index, x, W):
    out, _ = _run(edge_index, x, W)
    return out


# revision 11
# speedup vs baseline: 1.0431x; 1.0139x over previous
"""Trainium2 Bass kernel for nn_MessagePassing_9887014715655 (gnn_message_passing).

Reference computes:
    target   = edge_index[1]
    messages = x[target] * W[:, None]          # gather on target
    aggr     = segment_sum(messages, target)   # scatter on the SAME target

Because the gather index and the scatter index are identical, every message
for node n is x[n] * W[e], so

    aggr[n] = x[n] * s[n],   s = segment_sum(W, target)   # [N] weighted degree

The kernel therefore needs a weighted histogram of W over targets plus an
elementwise scale of x — a purely memory-bound problem (target_regime=memory).

Distribution strategy (chosen; the hint's edge-parallel+allreduce is strictly
worse here): the host performs LAYOUT ONLY — it stable-sorts edges by target
and scatters W into a zero-padded dense per-node matrix Wpad[n, :deg(n)], then
range-shards nodes across the 8 cores.  ALL floating-point arithmetic (the
per-node segment sums and the x*s scale) happens on device: each core
row-reduces its Wpad shard and multiplies its x shard by the result.
Node-range sharding makes each core's output independent, so no collective is
needed; each core streams Wpad (2.0MB) + x (1.6MB) in and out (1.6MB) at HBM
rate, double-buffered so DVE compute hides under DMA.
"""

import numpy as np

import concourse.bass as bass
import concourse.mybir as mybir
from concourse.bass_utils import run_bass_kernel_spmd

P = 128            # SBUF partitions
D = 32             # feature dim
N_CORES = 8
N_NODES = 100000
G = 98             # node-column groups per core; P*G*N_CORES = 100352 >= N_NODES
NPC = P * G        # nodes per core (12544)
N_PAD = NPC * N_CORES
F32 = mybir.dt.float32

_cache: dict = {}


def _build(K: int, n_chunks: int):
    """Per-core SPMD program (raw bass, manual semaphores).

    DRAM inputs (host pre-permuted so each partition's data is contiguous):
      wpad [P, G*K]  partition p, column-group g holds W-pad row of node g*P+p
      xin  [P, G*D]  same node layout for x
    Output:
      out  [P, G*D]  out[p, g*D:(g+1)*D] = xin[p, ...] * sum(wpad[p, g*K:(g+1)*K])
    """
    key = (K, n_chunks)
    if key in _cache:
        return _cache[key]

    # Skip bass's all-engine EVSEM barriers (module init + Block exit): our
    # first DMA (HWDGE on SP) has no dependency on the Pool const-memsets the
    # init barrier fences, and the final dout wait already fences the output
    # stores, so the exit barrier only adds EVSEM latency (~7us measured).
    _orig_barrier = bass.Bass.all_engine_barrier
    bass.Bass.all_engine_barrier = lambda self, **kw: None
    try:
        nc = _build_module(K, n_chunks)
    finally:
        bass.Bass.all_engine_barrier = _orig_barrier
    _cache[key] = nc
    return nc


def _build_module(K: int, n_chunks: int):
    nc = bass.Bass()
    wpad = nc.declare_dram_parameter("wpad", [P, G * K], F32, isOutput=False)
    xin = nc.declare_dram_parameter("xin", [P, G * D], F32, isOutput=False)
    out = nc.declare_dram_parameter("out", [P, G * D], F32, isOutput=True)

    CH = n_chunks
    base = G // CH
    sizes = [base + (1 if i < G % CH else 0) for i in range(CH)]
    offs = [sum(sizes[:i]) for i in range(CH)]
    gmax = max(sizes)

    import contextlib

    with contextlib.ExitStack() as ctx:
        wbuf = [
            ctx.enter_context(nc.sbuf_tensor(f"wbuf{j}", [P, gmax * K], F32))
            for j in range(2)
        ]
        xbuf = [
            ctx.enter_context(nc.sbuf_tensor(f"xbuf{j}", [P, gmax * D], F32))
            for j in range(2)
        ]
        obuf = [
            ctx.enter_context(nc.sbuf_tensor(f"obuf{j}", [P, gmax * D], F32))
            for j in range(2)
        ]
        sbuf = [
            ctx.enter_context(nc.sbuf_tensor(f"sbuf{j}", [P, gmax], F32))
            for j in range(2)
        ]
        # Separate sems per input stream: a shared sem would be racy — the 16
        # SDMA engines' +1 completions from concurrent DMAs interleave, so a
        # combined count can hit 32i+16 before the w-load fully landed.
        dinw = ctx.enter_context(nc.semaphore("dinw"))
        dinx = ctx.enter_context(nc.semaphore("dinx"))
        dout = ctx.enter_context(nc.semaphore("dout"))
        vd = ctx.enter_context(nc.semaphore("vd"))
        block = ctx.enter_context(nc.Block(no_gpsimd_drain=True))

        @block.sync
        def _(sync):
            # stream: L0, L1, S0, L2, S1, ..., L_{CH-1}, S_{CH-2}, S_{CH-1}
            def load(i):
                b = i % 2
                gc, g0 = sizes[i], offs[i]
                if i >= 2:
                    # WAR: wbuf/xbuf[b] are free once chunk i-2's mult retired
                    sync.wait_ge(vd, i - 1)
                sync.dma_start(
                    out=wbuf[b][:, : gc * K], in_=wpad[:, g0 * K:(g0 + gc) * K]
                ).then_inc(dinw, 16)
                sync.dma_start(
                    out=xbuf[b][:, : gc * D], in_=xin[:, g0 * D:(g0 + gc) * D]
                ).then_inc(dinx, 16)

            def store(i):
                b = i % 2
                gc, g0 = sizes[i], offs[i]
                sync.wait_ge(vd, i + 1)
                sync.dma_start(
                    out=out[:, g0 * D:(g0 + gc) * D], in_=obuf[b][:, : gc * D]
                ).then_inc(dout, 16)

            load(0)
            if CH > 1:
                load(1)
            for i in range(CH):
                if i + 2 < CH:
                    load(i + 2)
                store(i)
            sync.wait_ge(dout, 16 * CH)

        @block.vector
        def _(vector):
            for i in range(CH):
                b = i % 2
                gc = sizes[i]
                vector.wait_ge(dinw, 16 * i + 16)  # w_i fully landed
                vector.tensor_reduce(
                    out=sbuf[b][:, :gc],
                    in_=wbuf[b][:, : gc * K].rearrange("p (g k) -> p g k", k=K),
                    axis=mybir.AxisListType.X,
                    op=mybir.AluOpType.add,
                )
                vector.wait_ge(dinx, 16 * i + 16)  # x_i fully landed
                if i >= 2:
                    vector.wait_ge(dout, 16 * (i - 1))  # obuf[b] stored out
                vector.tensor_tensor(
                    out=obuf[b][:, : gc * D].rearrange("p (g d) -> p g d", d=D),
                    in0=xbuf[b][:, : gc * D].rearrange("p (g d) -> p g d", d=D),
                    in1=sbuf[b][:, :gc].unsqueeze(2).to_broadcast([P, gc, D]),
                    op=mybir.AluOpType.mult,
                ).then_inc(vd, 1)

    return nc


def _shard(a_padded: np.ndarray, width: int, core: int) -> np.ndarray:
    """[N_PAD, width] row-major -> this core's [P, G*width] partition-major."""
    sl = a_padded[core * NPC:(core + 1) * NPC]
    return np.ascontiguousarray(
        sl.reshape(G, P, width).transpose(1, 0, 2).reshape(P, G * width)
    )


def _prep(edge_index, x, W):
    """Host-side layout (integer metadata + data movement only, no FP math)."""
    t = np.asarray(edge_index)[1].astype(np.int64)
    x = np.ascontiguousarray(np.asarray(x, dtype=np.float32))
    W = np.ascontiguousarray(np.asarray(W, dtype=np.float32))
    n_nodes = x.shape[0]
    assert n_nodes <= N_PAD and x.shape[1] == D

    cnt = np.bincount(t, minlength=N_PAD)
    K = max(40, int(-(-int(cnt.max()) // 8) * 8))
    order = np.argsort(t, kind="stable")
    ts = t[order]
    Ws = W[order]
    starts = np.zeros(N_PAD, dtype=np.int64)
    starts[1:] = np.cumsum(cnt)[:-1]
    pos = np.arange(ts.shape[0], dtype=np.int64) - starts[ts]
    Wpad = np.zeros((N_PAD, K), dtype=np.float32)
    Wpad[ts, pos] = Ws
    xpad = np.zeros((N_PAD, D), dtype=np.float32)
    xpad[:n_nodes] = x

    in_maps = [
        {"wpad": _shard(Wpad, K, c), "xin": _shard(xpad, D, c)}
        for c in range(N_CORES)
    ]
    return in_maps, K, n_nodes


def _assemble(results, n_nodes):
    outs = []
    for c in range(N_CORES):
        oc = results[c]["out"].reshape(P, G, D).transpose(1, 0, 2)
        outs.append(oc.reshape(NPC, D))
    full = np.concatenate(outs, axis=0)[:n_nodes]
    return np.ascontiguousarray(full, dtype=np.float32)


def _run(edge_index, x, W, trace=False, n_chunks=2):
    in_maps, K, n_nodes = _prep(edge_index, x, W)
    nc = _build(K, n_chunks)
    res = run_bass_kernel_spmd(nc, in_maps, list(range(N_CORES)), trace=trace)
    return _assemble(res.results, n_nodes), res


def kernel(edge_index, x, W):
    out, _ = _run(edge_index, x, W)
    return out


# revision 13
# speedup vs baseline: 1.0656x; 1.0216x over previous
"""Trainium2 Bass kernel for nn_MessagePassing_9887014715655 (gnn_message_passing).

Reference computes:
    target   = edge_index[1]
    messages = x[target] * W[:, None]          # gather on target
    aggr     = segment_sum(messages, target)   # scatter on the SAME target

Because the gather index and the scatter index are identical, every message
for node n is x[n] * W[e], so

    aggr[n] = x[n] * s[n],   s = segment_sum(W, target)   # [N] weighted degree

The kernel therefore needs a weighted histogram of W over targets plus an
elementwise scale of x — a purely memory-bound problem (target_regime=memory).

Distribution strategy (chosen; the hint's edge-parallel+allreduce is strictly
worse here): the host performs LAYOUT ONLY — integer metadata and data
movement, no FP arithmetic.  Edges are stable-sorted by target; each core owns
a contiguous node range; within each core, nodes are sorted by degree
(descending) and mapped to (partition, column) = (j % 128, j // 128).  Each
128-node column's weight lists are zero-padded only to that column's own max
degree (rounded up to 4), so the banded weight buffer is ~E/8 bytes per core
instead of N*maxdeg.  Columns with equal padded width form runs, and one
strided tensor_reduce per run computes the per-node segment sums.

ALL floating-point arithmetic happens on device: the per-run reduces ARE the
segment sums (same edge order as the reference), then each core multiplies its
x shard by the result.  Node-range sharding makes each core's output
independent, so no collective is needed; per core the DMA stream is the banded
weights (~0.9MB) + x (1.6MB) in and out (1.6MB), x double-buffered, DVE
compute hidden under DMA, stores on the ACT HWDGE ring overlapping the SP
load ring.
"""

import contextlib

import numpy as np

import concourse.bass as bass
import concourse.mybir as mybir
from concourse.bass_utils import run_bass_kernel_spmd

P = 128            # SBUF partitions
D = 32             # feature dim
N_CORES = 8
N_NODES = 100000
G = 98             # node-column groups per core; P*G*N_CORES = 100352 >= N_NODES
NPC = P * G        # nodes per core (12544)
N_PAD = NPC * N_CORES
F32 = mybir.dt.float32

_cache: dict = {}


def _build(runs: tuple, n_chunks: int):
    """runs = ((n_cols, K), ...): consecutive column groups sharing padded
    width K.  sum(n_cols) == G.  The layout is identical on every core (host
    pads per-column widths to the max across cores) so one SPMD program
    serves all 8 cores."""
    key = (tuple(runs), n_chunks)
    if key in _cache:
        return _cache[key]

    # Skip bass's all-engine EVSEM barriers (module init + Block exit): our
    # first DMA (HWDGE on SP) has no dependency on the Pool const-memsets the
    # init barrier fences, and the final dout wait already fences the output
    # stores, so the exit barrier only adds EVSEM latency (~7us measured).
    _orig_barrier = bass.Bass.all_engine_barrier
    bass.Bass.all_engine_barrier = lambda self, **kw: None
    try:
        nc = _build_module(runs, n_chunks)
    finally:
        bass.Bass.all_engine_barrier = _orig_barrier
    _cache[key] = nc
    return nc


def _build_module(runs: tuple, n_chunks: int):
    nc = bass.Bass()
    C = int(sum(r * k for r, k in runs))     # banded buffer free-dim size

    wband = nc.declare_dram_parameter("wband", [P, C], F32, isOutput=False)
    xin = nc.declare_dram_parameter("xin", [P, G * D], F32, isOutput=False)
    out = nc.declare_dram_parameter("out", [P, G * D], F32, isOutput=True)

    CH = n_chunks
    base = G // CH
    sizes = [base + (1 if i < G % CH else 0) for i in range(CH)]
    offs = [sum(sizes[:i]) for i in range(CH)]
    gmax = max(sizes)

    with contextlib.ExitStack() as ctx:
        lbuf = ctx.enter_context(nc.sbuf_tensor("lbuf", [P, C], F32))
        st = ctx.enter_context(nc.sbuf_tensor("st", [P, G], F32))
        xbuf = [
            ctx.enter_context(nc.sbuf_tensor(f"xbuf{j}", [P, gmax * D], F32))
            for j in range(2)
        ]
        obuf = [
            ctx.enter_context(nc.sbuf_tensor(f"obuf{j}", [P, gmax * D], F32))
            for j in range(2)
        ]
        # one sem per DMA stream: completions of concurrent DMAs interleave
        # their 16 per-engine increments, so streams must not share a sem.
        dinw = ctx.enter_context(nc.semaphore("dinw"))
        dinx = ctx.enter_context(nc.semaphore("dinx"))
        dout = ctx.enter_context(nc.semaphore("dout"))
        vd = ctx.enter_context(nc.semaphore("vd"))
        vg = ctx.enter_context(nc.semaphore("vg"))
        block = ctx.enter_context(nc.Block(no_gpsimd_drain=True))

        @block.sync
        def _(sync):
            sync.dma_start(out=lbuf[:], in_=wband[:]).then_inc(dinw, 16)
            for i in range(CH):
                b = i % 2
                gc, g0 = sizes[i], offs[i]
                if i >= 2:
                    # WAR: xbuf[b] is free once chunk i-2's mult retired
                    sync.wait_ge(vd, i - 1)
                sync.dma_start(
                    out=xbuf[b][:, : gc * D], in_=xin[:, g0 * D:(g0 + gc) * D]
                ).then_inc(dinx, 16)

        @block.vector
        def _(vector):
            vector.memset(st[:], 0.0)          # zero-degree (padding) columns
            vector.wait_ge(dinw, 16)           # banded weights landed
            g0c = 0
            off = 0
            last = None
            for r, k in runs:
                if k > 0:
                    last = vector.tensor_reduce(
                        out=st[:, g0c:g0c + r],
                        in_=lbuf[:, off:off + r * k].rearrange(
                            "p (r k) -> p r k", k=k
                        ),
                        axis=mybir.AxisListType.X,
                        op=mybir.AluOpType.add,
                    )
                g0c += r
                off += r * k
            # same-engine RAW guard: sem fires only once the reduce's writes
            # are drained, so the mults below read a complete st.
            assert last is not None
            last.then_inc(vg, 1)
            for i in range(CH):
                b = i % 2
                gc, g0 = sizes[i], offs[i]
                vector.wait_ge(dinx, 16 * i + 16)   # x_i fully landed
                if i == 0:
                    vector.wait_ge(vg, 1)
                if i >= 2:
                    vector.wait_ge(dout, 16 * (i - 1))  # obuf[b] stored out
                vector.tensor_tensor(
                    out=obuf[b][:, : gc * D].rearrange("p (g d) -> p g d", d=D),
                    in0=xbuf[b][:, : gc * D].rearrange("p (g d) -> p g d", d=D),
                    in1=st[:, g0:g0 + gc].unsqueeze(2).to_broadcast([P, gc, D]),
                    op=mybir.AluOpType.mult,
                ).then_inc(vd, 1)

        @block.scalar
        def _(scalar):
            # stores ride the ACT HWDGE ring, overlapping the SP load ring
            for i in range(CH):
                b = i % 2
                gc, g0 = sizes[i], offs[i]
                scalar.wait_ge(vd, i + 1)
                scalar.dma_start(
                    out=out[:, g0 * D:(g0 + gc) * D], in_=obuf[b][:, : gc * D]
                ).then_inc(dout, 16)
            scalar.wait_ge(dout, 16 * CH)

    return nc


def _part_major(a: np.ndarray, width: int) -> np.ndarray:
    """[NPC, width] row-major -> [P, G*width] partition-major."""
    return np.ascontiguousarray(
        a.reshape(G, P, width).transpose(1, 0, 2).reshape(P, G * width)
    )


def _prep(edge_index, x, W):
    """Host-side layout (integer metadata + pure data movement, no FP math)."""
    t = np.asarray(edge_index)[1].astype(np.int64)
    x = np.ascontiguousarray(np.asarray(x, dtype=np.float32))
    W = np.ascontiguousarray(np.asarray(W, dtype=np.float32))
    n_nodes = x.shape[0]
    assert n_nodes <= N_PAD and x.shape[1] == D

    cnt = np.bincount(t, minlength=N_PAD)          # node degrees
    order_e = np.argsort(t, kind="stable")         # edges sorted by target
    Ws = W[order_e]
    starts = np.zeros(N_PAD, dtype=np.int64)
    starts[1:] = np.cumsum(cnt)[:-1]

    xpad = np.zeros((N_PAD, D), dtype=np.float32)
    xpad[:n_nodes] = x

    # per-core degree-descending node order; per-column max degree
    node_orders = []
    colmax = np.zeros((N_CORES, G), dtype=np.int64)
    for c in range(N_CORES):
        deg_c = cnt[c * NPC:(c + 1) * NPC]
        order_n = np.argsort(-deg_c, kind="stable")
        node_orders.append(order_n)
        sd = deg_c[order_n]
        colmax[c] = sd[::P][:G]                    # sorted desc: col max = first
    # shared per-column width across cores, rounded up to 4 (fewer runs)
    width = ((colmax.max(axis=0) + 3) // 4 * 4).astype(np.int64)
    runs = []
    for g in range(G):
        k = int(width[g])
        if runs and runs[-1][1] == k:
            runs[-1][0] += 1
        else:
            runs.append([1, k])
    runs = tuple((r, k) for r, k in runs)
    col_off = np.concatenate([[0], np.cumsum(width)]).astype(np.int64)
    C = int(col_off[-1])

    in_maps = []
    perms = []
    for c in range(N_CORES):
        order_n = node_orders[c]
        deg_c = cnt[c * NPC:(c + 1) * NPC][order_n]
        glob = c * NPC + order_n                   # global ids, degree-sorted
        band = np.zeros((P, C), dtype=np.float32)
        for g in range(G):
            k = int(width[g])
            if k == 0:
                continue
            nodes = glob[g * P:(g + 1) * P]        # 128 nodes of this column
            degs = deg_c[g * P:(g + 1) * P]
            # blk[p, j] = Ws[starts[nodes[p]] + j] for j < degs[p] else 0
            j = np.arange(k)[None, :]
            mask = j < degs[:, None]
            idx = starts[nodes][:, None] + j
            blk = np.where(mask, Ws[np.minimum(idx, len(Ws) - 1)], 0.0)
            band[:, col_off[g]:col_off[g + 1]] = blk
        xc = _part_major(xpad[glob], D)
        in_maps.append({"wband": band, "xin": xc})
        perms.append(glob)
    return in_maps, runs, perms, n_nodes


def _assemble(results, perms, n_nodes):
    full = np.zeros((N_PAD, D), dtype=np.float32)
    for c in range(N_CORES):
        oc = results[c]["out"].reshape(P, G, D).transpose(1, 0, 2).reshape(NPC, D)
        full[perms[c]] = oc
    return np.ascontiguousarray(full[:n_nodes], dtype=np.float32)


def _run(edge_index, x, W, trace=False, n_chunks=2):
    in_maps, runs, perms, n_nodes = _prep(edge_index, x, W)
    nc = _build(runs, n_chunks)
    res = run_bass_kernel_spmd(nc, in_maps, list(range(N_CORES)), trace=trace)
    return _assemble(res.results, perms, n_nodes), res


def kernel(edge_index, x, W):
    out, _ = _run(edge_index, x, W)
    return out


# revision 30
# speedup vs baseline: 1.2116x; 1.1370x over previous
"""Trainium2 Bass kernel for nn_MessagePassing_9887014715655 (gnn_message_passing).

Reference computes:
    target   = edge_index[1]
    messages = x[target] * W[:, None]          # gather on target
    aggr     = segment_sum(messages, target)   # scatter on the SAME target

Because the gather index and the scatter index are identical, every message
for node n is x[n] * W[e], so

    aggr[n] = x[n] * s[n],   s = segment_sum(W, target)   # [N] weighted degree

The kernel therefore needs a weighted histogram of W over targets plus an
elementwise scale of x — a purely memory-bound problem (target_regime=memory).

Distribution strategy (chosen; the hint's edge-parallel+allreduce is strictly
worse here): the host performs LAYOUT ONLY — integer metadata and data
movement, no FP arithmetic.  Edges are stable-sorted by target; each core owns
a contiguous node range; within each core, nodes are sorted by degree
(descending) and mapped to (partition, column) = (j % 128, j // 128).  Each
128-node column's weight lists are zero-padded only to that column's own max
degree (rounded up to 4), so the banded weight buffer is ~E/8 bytes per core
instead of N*maxdeg.  Columns with equal padded width form runs, and one
strided tensor_reduce per run computes the per-node segment sums.

ALL floating-point arithmetic happens on device: the per-run reduces ARE the
segment sums (same edge order as the reference), then each core multiplies its
x shard by the result.  Node-range sharding makes each core's output
independent, so no collective is needed; per core the DMA stream is the banded
weights (~0.9MB) + x (1.6MB) in and out (1.6MB), x double-buffered, DVE
compute hidden under DMA, stores on the ACT HWDGE ring overlapping the SP
load ring.
"""

import contextlib

import numpy as np

import concourse.bass as bass
import concourse.mybir as mybir
from concourse.bass_utils import run_bass_kernel_spmd

P = 128            # SBUF partitions
D = 32             # feature dim
N_CORES = 8
N_NODES = 100000
G = 98             # node-column groups per core; P*G*N_CORES = 100352 >= N_NODES
NPC = P * G        # nodes per core (12544)
N_PAD = NPC * N_CORES
F32 = mybir.dt.float32

_cache: dict = {}


def _build(runs: tuple, n_chunks: int, n_wsplit: int = 2, delay_x: bool = False, msplit: int = 1, xsizes: tuple | None = None):
    """runs = ((n_cols, K), ...): consecutive column groups sharing padded
    width K.  sum(n_cols) == G.  The layout is identical on every core (host
    pads per-column widths to the max across cores) so one SPMD program
    serves all 8 cores."""
    key = (tuple(runs), n_chunks, n_wsplit, delay_x, msplit, xsizes)
    if key in _cache:
        return _cache[key]

    # Skip bass's all-engine EVSEM barriers (module init + Block exit): our
    # first DMA (HWDGE on SP) has no dependency on the Pool const-memsets the
    # init barrier fences, and the final dout wait already fences the output
    # stores, so the exit barrier only adds EVSEM latency (~7us measured).
    _orig_barrier = bass.Bass.all_engine_barrier
    bass.Bass.all_engine_barrier = lambda self, **kw: None
    try:
        nc = _build_module(runs, n_chunks, n_wsplit, delay_x, msplit, xsizes)
    finally:
        bass.Bass.all_engine_barrier = _orig_barrier
    _cache[key] = nc
    return nc


def _build_module(runs: tuple, n_chunks: int, n_wsplit: int, delay_x: bool, msplit: int, xsizes: tuple | None):
    nc = bass.Bass()
    C = int(sum(r * k for r, k in runs))     # banded buffer free-dim size

    # split the run list into n_wsplit pieces of roughly equal bytes, at run
    # boundaries; each piece is one DMA + one reduce group
    WS = min(n_wsplit, len(runs))
    pieces: list = []          # list of list[(col0, off, r, k)]
    wsplit_cols: list = [0]    # band-offset boundaries per piece
    tgt = C / WS
    cur: list = []
    off = 0
    g0c = 0
    for r, k in runs:
        cur.append((g0c, off, r, k))
        off += r * k
        g0c += r
        if off >= tgt * len(wsplit_cols) and len(wsplit_cols) < WS:
            pieces.append(cur)
            cur = []
            wsplit_cols.append(off)
    pieces.append(cur)
    wsplit_cols.append(C)
    WS = len(pieces)

    wband = nc.declare_dram_parameter("wband", [P, C], F32, isOutput=False)
    xin = nc.declare_dram_parameter("xin", [P, G * D], F32, isOutput=False)
    out = nc.declare_dram_parameter("out", [P, G * D], F32, isOutput=True)

    CH = n_chunks
    if xsizes is not None:
        assert sum(xsizes) == G and len(xsizes) == CH
        sizes = list(xsizes)
    else:
        base = G // CH
        sizes = [base + (1 if i < G % CH else 0) for i in range(CH)]
    offs = [sum(sizes[:i]) for i in range(CH)]
    gmax = max(sizes)
    # mult/store sub-pieces within each chunk (smaller final store tail)
    MS = msplit
    sub: list = []     # (chunk, g0_abs, gc, g0_rel)
    for i in range(CH):
        mb = sizes[i] // MS
        ss = [mb + (1 if j < sizes[i] % MS else 0) for j in range(MS)]
        r0 = 0
        for j in range(MS):
            sub.append((i, offs[i] + r0, ss[j], r0))
            r0 += ss[j]
    NPIECE = len(sub)

    with contextlib.ExitStack() as ctx:
        lbuf = ctx.enter_context(nc.sbuf_tensor("lbuf", [P, C], F32))
        st = ctx.enter_context(nc.sbuf_tensor("st", [P, G], F32))
        xbuf = [
            ctx.enter_context(nc.sbuf_tensor(f"xbuf{j}", [P, gmax * D], F32))
            for j in range(2)
        ]
        obuf = [
            ctx.enter_context(nc.sbuf_tensor(f"obuf{j}", [P, gmax * D], F32))
            for j in range(2)
        ]
        # ONE SEM PER DMA INSTRUCTION: the 16 per-engine completion increments
        # of concurrent DMAs interleave arbitrarily, so any wait on a shared
        # sem below its final total can fire before the intended transfer has
        # fully landed.  Each DMA gets its own sem, waited at exactly 16.
        dinw = [
            ctx.enter_context(nc.semaphore(f"dinw{j}")) for j in range(WS)
        ]
        dinx = [
            ctx.enter_context(nc.semaphore(f"dinx{i}")) for i in range(CH)
        ]
        dout = [
            ctx.enter_context(nc.semaphore(f"dout{i}")) for i in range(NPIECE)
        ]
        vd = ctx.enter_context(nc.semaphore("vd"))
        vg = ctx.enter_context(nc.semaphore("vg"))
        block = ctx.enter_context(nc.Block(no_gpsimd_drain=True))

        @block.sync
        def _(sync):
            for j in range(WS):
                c0, c1 = wsplit_cols[j], wsplit_cols[j + 1]
                sync.dma_start(
                    out=lbuf[:, c0:c1], in_=wband[:, c0:c1]
                ).then_inc(dinw[j], 16)
            if delay_x:
                # give the band pieces the full SDMA round-robin share
                for j in range(WS):
                    sync.wait_ge(dinw[j], 16)
            for i in range(CH):
                b = i % 2
                gc, g0 = sizes[i], offs[i]
                if i >= 2:
                    # WAR: xbuf[b] is free once chunk i-2's mult retired
                    sync.wait_ge(vd, i - 1)
                sync.dma_start(
                    out=xbuf[b][:, : gc * D], in_=xin[:, g0 * D:(g0 + gc) * D]
                ).then_inc(dinx[i], 16)

        @block.vector
        def _(vector):
            vector.memset(st[:], 0.0)          # zero-degree (padding) columns
            last = None
            for j, piece in enumerate(pieces):
                vector.wait_ge(dinw[j], 16)    # this band piece landed
                for g0c, off, r, k in piece:
                    if k > 0:
                        last = vector.tensor_reduce(
                            out=st[:, g0c:g0c + r],
                            in_=lbuf[:, off:off + r * k].rearrange(
                                "p (r k) -> p r k", k=k
                            ),
                            axis=mybir.AxisListType.X,
                            op=mybir.AluOpType.add,
                        )
            # same-engine RAW guard: sem fires only once the reduce's writes
            # are drained, so the mults below read a complete st.
            assert last is not None
            last.then_inc(vg, 1)
            seen_chunk = -1
            for pi, (i, g0, gc, g0r) in enumerate(sub):
                b = i % 2
                if i != seen_chunk:
                    seen_chunk = i
                    vector.wait_ge(dinx[i], 16)    # x_i fully landed
                    if i == 0:
                        vector.wait_ge(vg, 1)
                    if i >= 2:
                        # obuf[b] free once chunk i-2's pieces all stored
                        for pj, (i2, _, _, _) in enumerate(sub):
                            if i2 == i - 2:
                                vector.wait_ge(dout[pj], 16)
                vector.tensor_tensor(
                    out=obuf[b][:, g0r * D:(g0r + gc) * D].rearrange(
                        "p (g d) -> p g d", d=D),
                    in0=xbuf[b][:, g0r * D:(g0r + gc) * D].rearrange(
                        "p (g d) -> p g d", d=D),
                    in1=st[:, g0:g0 + gc].unsqueeze(2).to_broadcast([P, gc, D]),
                    op=mybir.AluOpType.mult,
                ).then_inc(vd, 1)

        @block.scalar
        def _(scalar):
            # stores ride the ACT HWDGE ring, overlapping the SP load ring
            for pi, (i, g0, gc, g0r) in enumerate(sub):
                b = i % 2
                scalar.wait_ge(vd, pi + 1)
                scalar.dma_start(
                    out=out[:, g0 * D:(g0 + gc) * D],
                    in_=obuf[b][:, g0r * D:(g0r + gc) * D],
                ).then_inc(dout[pi], 16)
            for pi in range(NPIECE):
                scalar.wait_ge(dout[pi], 16)

    return nc


def _part_major(a: np.ndarray, width: int) -> np.ndarray:
    """[NPC, width] row-major -> [P, G*width] partition-major."""
    return np.ascontiguousarray(
        a.reshape(G, P, width).transpose(1, 0, 2).reshape(P, G * width)
    )


def _prep(edge_index, x, W):
    """Host-side layout (integer metadata + pure data movement, no FP math)."""
    t = np.asarray(edge_index)[1].astype(np.int64)
    x = np.ascontiguousarray(np.asarray(x, dtype=np.float32))
    W = np.ascontiguousarray(np.asarray(W, dtype=np.float32))
    n_nodes = x.shape[0]
    assert n_nodes <= N_PAD and x.shape[1] == D

    cnt = np.bincount(t, minlength=N_PAD)          # node degrees
    order_e = np.argsort(t, kind="stable")         # edges sorted by target
    Ws = W[order_e]
    starts = np.zeros(N_PAD, dtype=np.int64)
    starts[1:] = np.cumsum(cnt)[:-1]

    xpad = np.zeros((N_PAD, D), dtype=np.float32)
    xpad[:n_nodes] = x

    # per-core degree-descending node order; per-column max degree
    node_orders = []
    colmax = np.zeros((N_CORES, G), dtype=np.int64)
    for c in range(N_CORES):
        deg_c = cnt[c * NPC:(c + 1) * NPC]
        order_n = np.argsort(-deg_c, kind="stable")
        node_orders.append(order_n)
        sd = deg_c[order_n]
        colmax[c] = sd[::P][:G]                    # sorted desc: col max = first
    # shared per-column width across cores, rounded up to 4 (fewer runs)
    width = ((colmax.max(axis=0) + 3) // 4 * 4).astype(np.int64)
    runs = []
    for g in range(G):
        k = int(width[g])
        if runs and runs[-1][1] == k:
            runs[-1][0] += 1
        else:
            runs.append([1, k])
    runs = tuple((r, k) for r, k in runs)
    col_off = np.concatenate([[0], np.cumsum(width)]).astype(np.int64)
    C = int(col_off[-1])

    in_maps = []
    perms = []
    for c in range(N_CORES):
        order_n = node_orders[c]
        deg_c = cnt[c * NPC:(c + 1) * NPC][order_n]
        glob = c * NPC + order_n                   # global ids, degree-sorted
        band = np.zeros((P, C), dtype=np.float32)
        for g in range(G):
            k = int(width[g])
            if k == 0:
                continue
            nodes = glob[g * P:(g + 1) * P]        # 128 nodes of this column
            degs = deg_c[g * P:(g + 1) * P]
            # blk[p, j] = Ws[starts[nodes[p]] + j] for j < degs[p] else 0
            j = np.arange(k)[None, :]
            mask = j < degs[:, None]
            idx = starts[nodes][:, None] + j
            blk = np.where(mask, Ws[np.minimum(idx, len(Ws) - 1)], 0.0)
            band[:, col_off[g]:col_off[g + 1]] = blk
        xc = _part_major(xpad[glob], D)
        in_maps.append({"wband": band, "xin": xc})
        perms.append(glob)
    return in_maps, runs, perms, n_nodes


def _assemble(results, perms, n_nodes):
    full = np.zeros((N_PAD, D), dtype=np.float32)
    for c in range(N_CORES):
        oc = results[c]["out"].reshape(P, G, D).transpose(1, 0, 2).reshape(NPC, D)
        full[perms[c]] = oc
    return np.ascontiguousarray(full[:n_nodes], dtype=np.float32)


def _run(edge_index, x, W, trace=False, n_chunks=2, n_wsplit=1, delay_x=False,
         msplit=2, xsizes=None):
    in_maps, runs, perms, n_nodes = _prep(edge_index, x, W)
    nc = _build(runs, n_chunks, n_wsplit, delay_x, msplit, xsizes)
    res = run_bass_kernel_spmd(nc, in_maps, list(range(N_CORES)), trace=trace)
    return _assemble(res.results, perms, n_nodes), res


def kernel(edge_index, x, W):
    out, _ = _run(edge_index, x, W)
    return out


# revision 31
# speedup vs baseline: 1.2297x; 1.0149x over previous
"""Trainium2 Bass kernel for nn_MessagePassing_9887014715655 (gnn_message_passing).

Reference computes:
    target   = edge_index[1]
    messages = x[target] * W[:, None]          # gather on target
    aggr     = segment_sum(messages, target)   # scatter on the SAME target

Because the gather index and the scatter index are identical, every message
for node n is x[n] * W[e], so

    aggr[n] = x[n] * s[n],   s = segment_sum(W, target)   # [N] weighted degree

The kernel therefore needs a weighted histogram of W over targets plus an
elementwise scale of x — a purely memory-bound problem (target_regime=memory).

Distribution strategy (chosen; the hint's edge-parallel+allreduce is strictly
worse here): the host performs LAYOUT ONLY — integer metadata and data
movement, no FP arithmetic.  Edges are stable-sorted by target; each core owns
a contiguous node range; within each core, nodes are sorted by degree
(descending) and mapped to (partition, column) = (j % 128, j // 128).  Each
128-node column's weight lists are zero-padded only to that column's own max
degree (rounded up to 4), so the banded weight buffer is ~E/8 bytes per core
instead of N*maxdeg.  Columns with equal padded width form runs, and one
strided tensor_reduce per run computes the per-node segment sums.

ALL floating-point arithmetic happens on device: the per-run reduces ARE the
segment sums (same edge order as the reference), then each core multiplies its
x shard by the result.  Node-range sharding makes each core's output
independent, so no collective is needed; per core the DMA stream is the banded
weights (~0.9MB) + x (1.6MB) in and out (1.6MB), x double-buffered, DVE
compute hidden under DMA, stores on the ACT HWDGE ring overlapping the SP
load ring.
"""

import contextlib

import numpy as np

import concourse.bass as bass
import concourse.mybir as mybir
from concourse.bass_utils import run_bass_kernel_spmd

P = 128            # SBUF partitions
D = 32             # feature dim
N_CORES = 8
N_NODES = 100000
G = 98             # node-column groups per core; P*G*N_CORES = 100352 >= N_NODES
NPC = P * G        # nodes per core (12544)
N_PAD = NPC * N_CORES
F32 = mybir.dt.float32

_cache: dict = {}


def _build(runs: tuple, n_chunks: int, n_wsplit: int = 2, delay_x: bool = False, msplit: int = 1, xsizes: tuple | None = None):
    """runs = ((n_cols, K), ...): consecutive column groups sharing padded
    width K.  sum(n_cols) == G.  The layout is identical on every core (host
    pads per-column widths to the max across cores) so one SPMD program
    serves all 8 cores."""
    key = (tuple(runs), n_chunks, n_wsplit, delay_x, msplit, xsizes)
    if key in _cache:
        return _cache[key]

    # Skip bass's all-engine EVSEM barriers (module init + Block exit): our
    # first DMA (HWDGE on SP) has no dependency on the Pool const-memsets the
    # init barrier fences, and the final dout wait already fences the output
    # stores, so the exit barrier only adds EVSEM latency (~7us measured).
    _orig_barrier = bass.Bass.all_engine_barrier
    bass.Bass.all_engine_barrier = lambda self, **kw: None
    try:
        nc = _build_module(runs, n_chunks, n_wsplit, delay_x, msplit, xsizes)
    finally:
        bass.Bass.all_engine_barrier = _orig_barrier
    _cache[key] = nc
    return nc


def _build_module(runs: tuple, n_chunks: int, n_wsplit: int, delay_x: bool, msplit: int, xsizes: tuple | None):
    nc = bass.Bass()
    C = int(sum(r * k for r, k in runs))     # banded buffer free-dim size

    # split the run list into n_wsplit pieces of roughly equal bytes, at run
    # boundaries; each piece is one DMA + one reduce group
    WS = min(n_wsplit, len(runs))
    pieces: list = []          # list of list[(col0, off, r, k)]
    wsplit_cols: list = [0]    # band-offset boundaries per piece
    tgt = C / WS
    cur: list = []
    off = 0
    g0c = 0
    for r, k in runs:
        cur.append((g0c, off, r, k))
        off += r * k
        g0c += r
        if off >= tgt * len(wsplit_cols) and len(wsplit_cols) < WS:
            pieces.append(cur)
            cur = []
            wsplit_cols.append(off)
    pieces.append(cur)
    wsplit_cols.append(C)
    WS = len(pieces)

    wband = nc.declare_dram_parameter("wband", [P, C], F32, isOutput=False)
    xin = nc.declare_dram_parameter("xin", [P, G * D], F32, isOutput=False)
    out = nc.declare_dram_parameter("out", [P, G * D], F32, isOutput=True)

    CH = n_chunks
    if xsizes is not None:
        assert sum(xsizes) == G and len(xsizes) == CH
        sizes = list(xsizes)
    else:
        base = G // CH
        sizes = [base + (1 if i < G % CH else 0) for i in range(CH)]
    offs = [sum(sizes[:i]) for i in range(CH)]
    gmax = max(sizes)
    # mult/store sub-pieces within each chunk (smaller final store tail);
    # negative msplit = tapered: last sub-piece of each chunk is small so the
    # final store (the kernel tail) is short.
    MS = abs(msplit)
    tapered = msplit < 0
    sub: list = []     # (chunk, g0_abs, gc, g0_rel)
    for i in range(CH):
        if tapered and MS == 2 and sizes[i] > 24:
            ss = [sizes[i] - 12, 12]
        else:
            mb = sizes[i] // MS
            ss = [mb + (1 if j < sizes[i] % MS else 0) for j in range(MS)]
        r0 = 0
        for j in range(MS):
            sub.append((i, offs[i] + r0, ss[j], r0))
            r0 += ss[j]
    NPIECE = len(sub)

    with contextlib.ExitStack() as ctx:
        lbuf = ctx.enter_context(nc.sbuf_tensor("lbuf", [P, C], F32))
        st = ctx.enter_context(nc.sbuf_tensor("st", [P, G], F32))
        xbuf = [
            ctx.enter_context(nc.sbuf_tensor(f"xbuf{j}", [P, gmax * D], F32))
            for j in range(2)
        ]
        obuf = [
            ctx.enter_context(nc.sbuf_tensor(f"obuf{j}", [P, gmax * D], F32))
            for j in range(2)
        ]
        # ONE SEM PER DMA INSTRUCTION: the 16 per-engine completion increments
        # of concurrent DMAs interleave arbitrarily, so any wait on a shared
        # sem below its final total can fire before the intended transfer has
        # fully landed.  Each DMA gets its own sem, waited at exactly 16.
        dinw = [
            ctx.enter_context(nc.semaphore(f"dinw{j}")) for j in range(WS)
        ]
        dinx = [
            ctx.enter_context(nc.semaphore(f"dinx{i}")) for i in range(CH)
        ]
        dout = [
            ctx.enter_context(nc.semaphore(f"dout{i}")) for i in range(NPIECE)
        ]
        vd = ctx.enter_context(nc.semaphore("vd"))
        vg = ctx.enter_context(nc.semaphore("vg"))
        block = ctx.enter_context(nc.Block(no_gpsimd_drain=True))

        @block.sync
        def _(sync):
            for j in range(WS):
                c0, c1 = wsplit_cols[j], wsplit_cols[j + 1]
                sync.dma_start(
                    out=lbuf[:, c0:c1], in_=wband[:, c0:c1]
                ).then_inc(dinw[j], 16)
            if delay_x:
                # give the band pieces the full SDMA round-robin share
                for j in range(WS):
                    sync.wait_ge(dinw[j], 16)
            for i in range(CH):
                b = i % 2
                gc, g0 = sizes[i], offs[i]
                if i >= 2:
                    # WAR: xbuf[b] is free once chunk i-2's mult retired
                    sync.wait_ge(vd, i - 1)
                sync.dma_start(
                    out=xbuf[b][:, : gc * D], in_=xin[:, g0 * D:(g0 + gc) * D]
                ).then_inc(dinx[i], 16)

        @block.vector
        def _(vector):
            vector.memset(st[:], 0.0)          # zero-degree (padding) columns
            last = None
            for j, piece in enumerate(pieces):
                vector.wait_ge(dinw[j], 16)    # this band piece landed
                for g0c, off, r, k in piece:
                    if k > 0:
                        last = vector.tensor_reduce(
                            out=st[:, g0c:g0c + r],
                            in_=lbuf[:, off:off + r * k].rearrange(
                                "p (r k) -> p r k", k=k
                            ),
                            axis=mybir.AxisListType.X,
                            op=mybir.AluOpType.add,
                        )
            # same-engine RAW guard: sem fires only once the reduce's writes
            # are drained, so the mults below read a complete st.
            assert last is not None
            last.then_inc(vg, 1)
            seen_chunk = -1
            for pi, (i, g0, gc, g0r) in enumerate(sub):
                b = i % 2
                if i != seen_chunk:
                    seen_chunk = i
                    vector.wait_ge(dinx[i], 16)    # x_i fully landed
                    if i == 0:
                        vector.wait_ge(vg, 1)
                    if i >= 2:
                        # obuf[b] free once chunk i-2's pieces all stored
                        for pj, (i2, _, _, _) in enumerate(sub):
                            if i2 == i - 2:
                                vector.wait_ge(dout[pj], 16)
                vector.tensor_tensor(
                    out=obuf[b][:, g0r * D:(g0r + gc) * D].rearrange(
                        "p (g d) -> p g d", d=D),
                    in0=xbuf[b][:, g0r * D:(g0r + gc) * D].rearrange(
                        "p (g d) -> p g d", d=D),
                    in1=st[:, g0:g0 + gc].unsqueeze(2).to_broadcast([P, gc, D]),
                    op=mybir.AluOpType.mult,
                ).then_inc(vd, 1)

        @block.scalar
        def _(scalar):
            # stores ride the ACT HWDGE ring, overlapping the SP load ring
            for pi, (i, g0, gc, g0r) in enumerate(sub):
                b = i % 2
                scalar.wait_ge(vd, pi + 1)
                scalar.dma_start(
                    out=out[:, g0 * D:(g0 + gc) * D],
                    in_=obuf[b][:, g0r * D:(g0r + gc) * D],
                ).then_inc(dout[pi], 16)
            for pi in range(NPIECE):
                scalar.wait_ge(dout[pi], 16)

    return nc


def _part_major(a: np.ndarray, width: int) -> np.ndarray:
    """[NPC, width] row-major -> [P, G*width] partition-major."""
    return np.ascontiguousarray(
        a.reshape(G, P, width).transpose(1, 0, 2).reshape(P, G * width)
    )


def _prep(edge_index, x, W):
    """Host-side layout (integer metadata + pure data movement, no FP math)."""
    t = np.asarray(edge_index)[1].astype(np.int64)
    x = np.ascontiguousarray(np.asarray(x, dtype=np.float32))
    W = np.ascontiguousarray(np.asarray(W, dtype=np.float32))
    n_nodes = x.shape[0]
    assert n_nodes <= N_PAD and x.shape[1] == D

    cnt = np.bincount(t, minlength=N_PAD)          # node degrees
    order_e = np.argsort(t, kind="stable")         # edges sorted by target
    Ws = W[order_e]
    starts = np.zeros(N_PAD, dtype=np.int64)
    starts[1:] = np.cumsum(cnt)[:-1]

    xpad = np.zeros((N_PAD, D), dtype=np.float32)
    xpad[:n_nodes] = x

    # per-core degree-descending node order; per-column max degree
    node_orders = []
    colmax = np.zeros((N_CORES, G), dtype=np.int64)
    for c in range(N_CORES):
        deg_c = cnt[c * NPC:(c + 1) * NPC]
        order_n = np.argsort(-deg_c, kind="stable")
        node_orders.append(order_n)
        sd = deg_c[order_n]
        colmax[c] = sd[::P][:G]                    # sorted desc: col max = first
    # shared per-column width across cores, rounded up to 4 (fewer runs)
    width = ((colmax.max(axis=0) + 3) // 4 * 4).astype(np.int64)
    runs = []
    for g in range(G):
        k = int(width[g])
        if runs and runs[-1][1] == k:
            runs[-1][0] += 1
        else:
            runs.append([1, k])
    runs = tuple((r, k) for r, k in runs)
    col_off = np.concatenate([[0], np.cumsum(width)]).astype(np.int64)
    C = int(col_off[-1])

    in_maps = []
    perms = []
    for c in range(N_CORES):
        order_n = node_orders[c]
        deg_c = cnt[c * NPC:(c + 1) * NPC][order_n]
        glob = c * NPC + order_n                   # global ids, degree-sorted
        band = np.zeros((P, C), dtype=np.float32)
        for g in range(G):
            k = int(width[g])
            if k == 0:
                continue
            nodes = glob[g * P:(g + 1) * P]        # 128 nodes of this column
            degs = deg_c[g * P:(g + 1) * P]
            # blk[p, j] = Ws[starts[nodes[p]] + j] for j < degs[p] else 0
            j = np.arange(k)[None, :]
            mask = j < degs[:, None]
            idx = starts[nodes][:, None] + j
            blk = np.where(mask, Ws[np.minimum(idx, len(Ws) - 1)], 0.0)
            band[:, col_off[g]:col_off[g + 1]] = blk
        xc = _part_major(xpad[glob], D)
        in_maps.append({"wband": band, "xin": xc})
        perms.append(glob)
    return in_maps, runs, perms, n_nodes


def _assemble(results, perms, n_nodes):
    full = np.zeros((N_PAD, D), dtype=np.float32)
    for c in range(N_CORES):
        oc = results[c]["out"].reshape(P, G, D).transpose(1, 0, 2).reshape(NPC, D)
        full[perms[c]] = oc
    return np.ascontiguousarray(full[:n_nodes], dtype=np.float32)


def _run(edge_index, x, W, trace=False, n_chunks=2, n_wsplit=1, delay_x=False,
         msplit=2, xsizes=None):
    in_maps, runs, perms, n_nodes = _prep(edge_index, x, W)
    nc = _build(runs, n_chunks, n_wsplit, delay_x, msplit, xsizes)
    res = run_bass_kernel_spmd(nc, in_maps, list(range(N_CORES)), trace=trace)
    return _assemble(res.results, perms, n_nodes), res


def kernel(edge_index, x, W):
    out, _ = _run(edge_index, x, W)
    return out
